# revision 12
# baseline (speedup 1.0000x reference)
"""Trainium2 Bass kernel for nn_Coo2Cel (periodic pairwise displacement grid).

v2: compact minimum-image output + TensorEngine partition-broadcast input.

Reference semantics (B=1, N=1024 atoms, diagonal 30 A cell, rc=6):
  out[b,i,j,s,:] = (vec, sod), vec = pos_i - pos_j - shift_s, sod = |vec|^2,
  zeroed unless sod < rc^2 (self-pair at zero shift also zeroed).

Structure exploited: box=30 > 2*rc=12, so for every (i,j) pair AT MOST ONE
of the 27 shifts can pass the cutoff -- the minimum-image shift.  The device
computes for all pairs (bit-exact vs the f32 reference):
    n_c  = (d_c >= 15) - (d_c <= -15)    in {-1,0,1}
    w_c  = d_c - 30*n_c                  (one STT: 30*m + d, m = -n)
    sod  = (wx^2 + wy^2) + wz^2          (same eval order as reference)
    sneg = 9*mx + 3*my + mz              (shift code; host: s = 13 - sneg)
emitting five [P, N] f32 planes per core (2.6 MB vs 56.6 MB dense).  Host
zero-fills the dense tensor and scatters rows with sod < rc^2.

Input avoids the 1.57 MB per-core replicated candidate load: candidates are
DMA'd once as a [1, 3N] row (12 KB) and broadcast across the 128 partitions
by the TensorEngine (ones[1,128].T @ p[1,512] per PSUM bank) -- a single
1.0*x product per element, which is exact.  DVE reads the candidate planes
directly from PSUM.

Engine split per tile: DVE d/m/w/sod (11 planes), GpSimd a/sneg (5 planes),
ScalarE squares (3 planes); output DMA on the sync (HWDGE) queue.
"""
import sys

if "/opt/trn_rl_repo" not in sys.path:
    sys.path.insert(0, "/opt/trn_rl_repo")

import numpy as np

N = 1024          # atoms
S = 27            # lattice shifts
P = 128           # partitions / query rows per core
NCORES = 8
RC2 = 36.0        # rc^2, rc = 6.0
JT = 256          # candidate tile size
NT = N // JT
BANK = 512        # PSUM bank size in f32

TRACE = False
LAST_RESULT = None

_CACHE = {}


def _build(box, pbc_tuple, repeat=1, inner=1):
    """repeat>1 wraps `inner` full passes in a hardware For_i loop of
    `repeat` iterations (bench-only; the graded path uses repeat=1)."""
    import concourse.bacc as bacc
    import concourse.mybir as mybir
    from concourse.tile import TileContext

    F32 = mybir.dt.float32
    ADD = mybir.AluOpType.add
    MULT = mybir.AluOpType.mult
    SUB = mybir.AluOpType.subtract
    ISGE = mybir.AluOpType.is_ge
    ISLE = mybir.AluOpType.is_le
    half = float(box[0]) * 0.5

    nc = bacc.Bacc()
    qin_d = nc.declare_dram_parameter("qin", [P, 3, 1], F32, isOutput=False)
    pT_d = nc.declare_dram_parameter("pT", [1, 3 * N], F32, isOutput=False)
    out_d = nc.declare_dram_parameter("out", [P, 5, N], F32, isOutput=True)

    with TileContext(nc) as tc:
        with (
            tc.tile_pool(name="const", bufs=1) as cpool,
            tc.tile_pool(name="ppsum", bufs=1, space="PSUM") as ppool,
            tc.tile_pool(name="work", bufs=2) as wpool,
            tc.tile_pool(name="outp", bufs=3) as opool,
        ):
            qin = cpool.tile([P, 3, 1], F32)
            pT = cpool.tile([1, 3 * N], F32)
            ones = cpool.tile([1, P], F32)
            nc.vector.memset(ones[:], 1.0)
            nc.sync.dma_start(out=qin[:], in_=qin_d[:])
            nc.sync.dma_start(out=pT[:], in_=pT_d[:])

            # candidates broadcast across partitions: PSUM [P, 3, N]
            pp = ppool.tile([P, 3, N], F32)
            ppf = pp[:].rearrange("p c n -> p (c n)")
            for b in range(3 * N // BANK):
                nc.tensor.matmul(
                    ppf[:, b * BANK:(b + 1) * BANK],
                    ones[:],
                    pT[:, b * BANK:(b + 1) * BANK],
                    start=True, stop=True,
                )

            def front(jt):
                """DVE-only chain + kick off ScalarE squares; no DVE op
                here waits on another engine."""
                js = slice(jt * JT, (jt + 1) * JT)
                d = wpool.tile([P, 3, JT], F32, tag="d")
                a = wpool.tile([P, 3, JT], F32, tag="a")
                m = wpool.tile([P, 3, JT], F32, tag="m")
                sq = wpool.tile([P, 3, JT], F32, tag="sq")
                t2 = wpool.tile([P, JT], F32, tag="t2")
                outt = opool.tile([P, 5, JT], F32, tag="outt")
                # d = q - p   (query column broadcast along j; p from PSUM)
                nc.vector.tensor_tensor(
                    out=d[:],
                    in0=qin[:].broadcast_to([P, 3, JT]),
                    in1=pp[:, :, js],
                    op=SUB,
                )
                # a = (d >= box/2)
                nc.vector.tensor_scalar(
                    out=a[:], in0=d[:], scalar1=half, scalar2=None,
                    op0=ISGE)
                # m = (d <= -box/2) - a   == -n
                nc.vector.scalar_tensor_tensor(
                    out=m[:], in0=d[:], scalar=-half, in1=a[:],
                    op0=ISLE, op1=SUB)
                for c in range(3):
                    if not pbc_tuple[c]:
                        nc.vector.memset(m[:, c, :], 0.0)
                # w = box*m + d   (minimum image, exact)
                nc.vector.scalar_tensor_tensor(
                    out=outt[:, 0:3, :], in0=m[:], scalar=float(box[0]),
                    in1=d[:], op0=MULT, op1=ADD)
                # squares on ScalarE (own SBUF port)
                nc.scalar.activation(
                    out=sq[:], in_=outt[:, 0:3, :],
                    func=mybir.ActivationFunctionType.Square)
                # sneg = 9*mx + (3*my + mz)
                nc.vector.scalar_tensor_tensor(
                    out=t2[:], in0=m[:, 1, :], scalar=3.0, in1=m[:, 2, :],
                    op0=MULT, op1=ADD)
                nc.vector.scalar_tensor_tensor(
                    out=outt[:, 4, :], in0=m[:, 0, :], scalar=9.0,
                    in1=t2[:], op0=MULT, op1=ADD)
                return js, sq, outt

            def back(state):
                """sod assembly (waits on ScalarE) + output DMA; issued a
                tile late so the wait is covered by the next front."""
                js, sq, outt = state
                t1 = wpool.tile([P, JT], F32, tag="t1")
                # sod = (sqx + sqy) + sqz  -- reference eval order
                nc.vector.tensor_tensor(
                    out=t1[:], in0=sq[:, 0, :], in1=sq[:, 1, :], op=ADD)
                nc.vector.tensor_tensor(
                    out=outt[:, 3, :], in0=t1[:], in1=sq[:, 2, :], op=ADD)
                nc.sync.dma_start(out=out_d[:, :, js], in_=outt[:])

            def one_pass():
                prev = None
                for jt in range(NT):
                    state = front(jt)
                    if prev is not None:
                        back(prev)
                    prev = state
                back(prev)

            if repeat > 1:
                with tc.For_i(0, repeat, 1):
                    for _ in range(inner):
                        one_pass()
            else:
                for _ in range(inner):
                    one_pass()
    nc.finalize()
    return nc


def _prepare(pos_cel, cel_mat, pbc):
    """Host-side shard prep: returns (box, pbc_tuple, in_maps)."""
    pos_cel = np.asarray(pos_cel)
    cel_mat = np.asarray(cel_mat, dtype=np.float32)
    pbc = np.asarray(pbc)
    B = pos_cel.shape[0]
    assert pos_cel.shape == (B, N, 3), pos_cel.shape
    assert B == 1

    pos = (pos_cel[0].astype(np.float32) @ cel_mat[0]).astype(np.float32)
    off = cel_mat[0] - np.diag(np.diag(cel_mat[0]))
    assert np.all(off == 0), "kernel assumes a diagonal cell matrix"
    box = tuple(float(cel_mat[0][c, c]) for c in range(3))
    assert box[0] == box[1] == box[2], "kernel assumes a cubic cell"
    pbc_tuple = tuple(bool(x) for x in pbc[0])

    pT = np.ascontiguousarray(pos.T).reshape(1, 3 * N)   # [1, 3N]
    in_maps = []
    for k in range(NCORES):
        qin = np.ascontiguousarray(pos[k * P:(k + 1) * P])[:, :, None]
        in_maps.append({"pT": pT, "qin": qin})
    return box, pbc_tuple, in_maps


def kernel(pos_cel, cel_mat, pbc):
    global LAST_RESULT
    from concourse.bass_utils import run_bass_kernel_spmd

    box, pbc_tuple, in_maps = _prepare(pos_cel, cel_mat, pbc)
    key = (box, pbc_tuple)
    if key not in _CACHE:
        _CACHE[key] = _build(box, pbc_tuple)
    nc = _CACHE[key]

    res = run_bass_kernel_spmd(nc, in_maps, list(range(NCORES)), trace=TRACE)
    LAST_RESULT = res

    out = np.zeros((1, N, N, S, 4), dtype=np.float32)
    flat = out.reshape(N * N * S, 4)
    for k in range(NCORES):
        arr = np.asarray(res.results[k]["out"]).reshape(P, 5, N)
        sod = arr[:, 3]
        ii, jj = np.nonzero(sod < RC2)
        sidx = 13 - arr[:, 4][ii, jj].astype(np.int64)
        gi = k * P + ii
        idx = (gi * N + jj) * S + sidx
        flat[idx, 0] = arr[:, 0][ii, jj]
        flat[idx, 1] = arr[:, 1][ii, jj]
        flat[idx, 2] = arr[:, 2][ii, jj]
        flat[idx, 3] = sod[ii, jj]
    return out
